# revision 15
# baseline (speedup 1.0000x reference)
"""Trainium2 Bass kernel v7 for nn_Middle_Moudle_v3 (retrieval_knn).

Per episode (b, s): cosine similarity of every support spatial C-vector
against every query spatial C-vector, max over query positions.

  support_x, query_x: [8, 75, 64, 19, 19] fp32  ->  out [8, 75, 361] fp32

Data-parallel over batch (8 episodes -> 8 cores).

Design (v7):
  - Query is normalized ON HOST (fp32) and shipped as q-hat bf16; support
    norms are applied on host after the kernel (scale commutes with max).
    No on-device norm pipeline at all.
  - HAM fix: K=64 half-array matmuls never register as PE activity (v2
    trace: clock stuck at 1.2 GHz the whole main phase).  The host ships
    the support TWICE with the opposite partition half zeroed (sup0:
    even pairs in partitions 0-63, zeros below; sup1: odd pairs in
    64-127, zeros above).  Every main matmul is then a genuine K=128
    full-array op -> HAM stays at 2.4 GHz (153ns/MM).
  - Drain split tuned from trace so both PSUM-capable engines run
    concurrently at equal load: per 12-pair span, P%4==0 direct-reduces
    on DVE from PSUM (1490ns); the other 9 are ACT-copied to SBUF bf16
    (1163ns) and max-folded on DVE (overlap-fold ladder, ~650ns/pair).
  - Input DMA spread over three HW queues (sync/tensor/vector issue) so
    each carries ~3.5MB and the first tile group lands early; ACT's
    queue carries no DMA triggers.
"""
import numpy as np
import ml_dtypes

import concourse.bass as bass
import concourse.mybir as mybir
import concourse.tile as tile
from concourse.bass_utils import run_bass_kernel_spmd

F32 = mybir.dt.float32
BF16 = mybir.dt.bfloat16
B = 8
S = 75
C = 64
N = 361
N2 = 362
SP = 76        # padded pairs
NT = 38        # two-pair tiles
NP = 75        # computed pairs (pad pair 75 skipped)
CHUNKS = [(0, 128), (128, 128), (256, 105)]
DGROUPS = [(0, 3), (3, 5), (8, 6), (14, 6), (20, 6), (26, 6), (32, 6)]
HEAT = 26      # K=128 heater matmuls to warm the PE clock during input DMA
EPS = 1e-8

SPANP = 12     # pairs per span
NSPAN = 6
# span drain mixes: B = 4 direct + 8 fold (DVE-heavy, early where DVE
# idles on DMA), A = 3 + 9 (balanced), C = 2 + 10 (ACT-heavy, late where
# ACT would idle); pairs 72-74 fold via ACT into its idle tail ('M').
SPAN_TYPES = ['B', 'B', 'A', 'A', 'A', 'C']

_ws_ctr = [0]


def _split_multi_waits(nc):
    """Move all-but-one sync wait of each instruction onto injected
    InstEventSemaphore instructions (standalone sequencer waits)."""
    for f in nc.m.functions:
        for bb in f.blocks:
            insts = list(bb.instructions)
            out = []
            changed = False
            for ins in insts:
                si = ins.sync_info
                if si is not None and len(si.on_wait) > 1:
                    waits = list(si.on_wait)
                    for w in waits[:-1]:
                        _ws_ctr[0] += 1
                        ev = mybir.InstEventSemaphore(
                            name=f"wsplit_{_ws_ctr[0]}",
                            engine=ins.engine,
                            sync_info=mybir.SyncInfo(on_wait=[w], on_update=[]),
                        )
                        out.append(ev)
                    ins.sync_info = mybir.SyncInfo(
                        on_wait=[waits[-1]], on_update=list(si.on_update)
                    )
                    changed = True
                out.append(ins)
            if changed:
                bb.instructions = out


def _build_nc():
    win_np = np.zeros((128, 152), dtype=np.float32)
    win_np[:, 74] = 1.0

    nc = bass.Bass(target_bir_lowering=False)
    sup0_d = nc.dram_tensor("support0", [64, NT * N2], BF16,
                            kind="ExternalInput")
    sup1_d = nc.dram_tensor("support1", [64, NT * N2], BF16,
                            kind="ExternalInput")
    qry_d = nc.dram_tensor("query", [128, NT * N2], BF16, kind="ExternalInput")
    cmax_d = nc.dram_tensor("cmax", [128, 3 * SP], F32, kind="ExternalOutput")
    win_d = nc.inline_tensor(win_np, name="win")

    mx = mybir.AluOpType.max

    with tile.TileContext(nc) as tc:
        with tc.tile_pool(name="inp", bufs=1) as inp, \
             tc.tile_pool(name="work", bufs=1) as work, \
             tc.tile_pool(name="fbp", bufs=2) as fbp, \
             tc.tile_pool(name="psd", bufs=1, space="PSUM") as psd:

            # heater weights (K=128) land first; heat the PE clock
            # while the big input DMAs stream in.
            win32 = work.tile([128, 152], F32)
            nc.sync.dma_start(win32[:], win_d[:])
            win_sb = work.tile([128, 152], BF16)
            nc.vector.tensor_copy(win_sb[:], win32[:])

            sup0 = inp.tile([128, NT, N2], BF16)
            sup1 = inp.tile([128, NT, N2], BF16)
            qt = inp.tile([128, NT, N2], BF16)

            # data halves by DMA; dead halves zeroed per-group on the
            # otherwise-idle GPSIMD engine (input bytes: 10.6 -> 7.0 MB).
            for (g0, T) in DGROUPS:
                a, b = g0 * N2, (g0 + T) * N2
                nc.gpsimd.memset(sup0[64:128, g0:g0 + T, :], 0.0)
                nc.gpsimd.memset(sup1[0:64, g0:g0 + T, :], 0.0)
                nc.sync.dma_start(sup0[0:64, g0:g0 + T, :], sup0_d[:, a:b])
                nc.scalar.dma_start(sup1[64:128, g0:g0 + T, :],
                                    sup1_d[:, a:b])
                nc.sync.dma_start(qt[:, g0:g0 + T, :], qry_d[:, a:b])

            colmax = work.tile([128, SP, 3], F32)

            # --- 8-bank chunk-slot rotation: chunk (P, m) lands in bank
            # (3P+m) %% 8, giving ~2.67 pairs of PSUM pipeline depth.
            dots8 = psd.tile([128, 8, 512], F32)

            # --- heater: full-array K=128 matmuls into bank 0 (WAW-
            # serialized ahead of pair 0's first chunk)
            for h in range(HEAT):
                nc.tensor.matmul(dots8[:, 0, 0:152], win_sb[:, 0:128],
                                 win_sb[:, 0:152], start=True, stop=True)

            pend = []

            def fold_chain(fb, s0, ns):
                """Overlap-fold ladder on fb slots [s0, s0+ns)."""
                steps = []
                for (hi, w) in ((180, 182), (90, 92), (46, 46), (22, 24)):
                    steps.append(lambda hi=hi, w=w: nc.vector.tensor_tensor(
                        out=fb[:, s0:s0 + ns, :, 0:w],
                        in0=fb[:, s0:s0 + ns, :, 0:w],
                        in1=fb[:, s0:s0 + ns, :, hi:hi + w], op=mx))
                return steps

            def tails(fb, bi, styp):
                """Strided tail reduces mapping fb slots to colmax."""
                P0 = SPANP * bi
                steps = []

                def tail(s0_, sstep, nf, pP0, pstride):
                    cv = colmax[:, pP0, :]
                    out_ap = bass.AP(tensor=cv.tensor, offset=cv.offset,
                                     ap=[list(cv.ap[0]), [3 * pstride, nf],
                                         [1, 3]])
                    v = fb[:, s0_:s0_ + 1, :, 0:24]
                    in_ap = bass.AP(tensor=v.tensor, offset=v.offset,
                                    ap=[list(v.ap[0]), [sstep * 3 * N2, nf],
                                        [N2, 3], [1, 24]])
                    nc.vector.tensor_reduce(
                        out_ap, in_ap, axis=mybir.AxisListType.X, op=mx)
                if styp == 'A':
                    spec = [(3 * t, 1, 3, P0 + 1 + t, 4) for t in range(3)]
                elif styp == 'B':
                    spec = [(4 * t, 1, 4, P0 + 1 + t, 3) for t in range(2)]
                elif styp == 'C':
                    spec = [(i, 5, 2, P0 + 1 + i, 6) for i in range(5)]
                else:   # M: pairs 72..74 in slots 0..2
                    spec = [(0, 1, 3, 72, 1)]
                for (a, b, c, dd, ee) in spec:
                    steps.append(lambda a=a, b=b, c=c, dd=dd, ee=ee:
                                 tail(a, b, c, dd, ee))
                return steps

            fb = None
            for j in range(NT):
                for e in range(2):
                    P = 2 * j + e
                    if P >= NP:
                        continue
                    supE = sup0 if e == 0 else sup1
                    s0 = (3 * P) % 8
                    for m, (off, mc) in enumerate(CHUNKS):
                        nc.tensor.matmul(
                            dots8[0:mc, (s0 + m) % 8, 0:N2],
                            supE[:, j, off:off + mc],
                            qt[:, j, 0:N2],
                            start=True, stop=True,
                        )
                    pj = P % SPANP
                    bi = P // SPANP
                    styp = SPAN_TYPES[bi] if bi < NSPAN else 'M'
                    if pj == 0 and bi < NSPAN:
                        fb = fbp.tile([128, 10, 3, N2], BF16, tag="fb",
                                      name=f"fb{bi}")
                    if styp == 'M' and P == 72:
                        fb = fbp.tile([128, 10, 3, N2], BF16, tag="fb",
                                      name="fb6")
                    if styp == 'A':
                        direct = (pj % 4 == 0)
                        slot = -1 if direct else 3 * (pj % 4 - 1) + pj // 4
                    elif styp == 'B':
                        direct = (pj % 3 == 0)
                        slot = -1 if direct else 4 * (pj % 3 - 1) + pj // 3
                    elif styp == 'C':
                        direct = (pj in (0, 6))
                        slot = -1 if direct else (pj - 1 if pj < 6 else pj - 2)
                    else:
                        direct = False
                        slot = P - 72
                    nw = min(3, 8 - s0)   # chunks before the bank wrap
                    if direct:
                        if nw == 3:
                            nc.vector.tensor_reduce(
                                colmax[:, P, :], dots8[:, s0:s0 + 3, 0:N2],
                                axis=mybir.AxisListType.X, op=mx)
                        else:
                            nc.vector.tensor_reduce(
                                colmax[:, P, 0:nw],
                                dots8[:, s0:s0 + nw, 0:N2],
                                axis=mybir.AxisListType.X, op=mx)
                            nc.vector.tensor_reduce(
                                colmax[:, P, nw:3],
                                dots8[:, 0:3 - nw, 0:N2],
                                axis=mybir.AxisListType.X, op=mx)
                    else:
                        if nw == 3:
                            nc.scalar.copy(fb[:, slot, :, :],
                                           dots8[:, s0:s0 + 3, 0:N2])
                        else:
                            nc.scalar.copy(fb[:, slot, 0:nw, :],
                                           dots8[:, s0:s0 + nw, 0:N2])
                            nc.scalar.copy(fb[:, slot, nw:3, :],
                                           dots8[:, 0:3 - nw, 0:N2])
                        if pj == SPANP - 1 and styp == 'A':
                            pend.extend(fold_chain(fb, 0, 9))
                            pend.extend(tails(fb, bi, 'A'))
                        elif pj == SPANP - 1 and styp == 'B':
                            pend.extend(fold_chain(fb, 0, 8))
                            pend.extend(tails(fb, bi, 'B'))
                        elif pj == SPANP - 1 and styp == 'C':
                            pend.extend(fold_chain(fb, 0, 10))
                            pend.extend(tails(fb, bi, 'C'))
                        elif styp == 'M' and P == NP - 1:
                            pend.extend(fold_chain(fb, 0, 3))
                            pend.extend(tails(fb, 6, 'M'))
                    if P == 48:
                        # all colmax[:, 0:36] writers are issued by now
                        nc.sync.dma_start(cmax_d[:, 0:108],
                                          colmax[:, 0:36, :])
                    if P == 72:
                        nc.sync.dma_start(cmax_d[:, 108:180],
                                          colmax[:, 36:60, :])
                    if pend and (pj >= 2 or bi >= NSPAN - 1):
                        pend.pop(0)()
                        if pend:
                            pend.pop(0)()
            for fn in pend:
                fn()

            nc.sync.dma_start(cmax_d[:, 180:3 * SP], colmax[:, 60:SP, :])

    _split_multi_waits(nc)
    return nc


_NC_CACHE = None


def _get_nc():
    global _NC_CACHE
    if _NC_CACHE is None:
        _NC_CACHE = _build_nc()
    return _NC_CACHE


def _pack(x):
    """[B, S, C, N] fp32 -> [B, 128, NT*N2] bf16 (pad dup)."""
    x = np.concatenate([x, x[:, S - 1:S]], axis=1)          # pair 75 = dup 74
    x = np.concatenate([x, x[:, :, :, N - 1:N]], axis=3)    # col 361 = dup 360
    x = x.reshape(B, NT, 2, C, N2).transpose(0, 2, 3, 1, 4)  # [B,2,C,NT,N2]
    x = x.reshape(B, 128, NT * N2)
    return np.ascontiguousarray(x.astype(ml_dtypes.bfloat16))


def _prep(support_x, query_x):
    sx = np.asarray(support_x, dtype=np.float32).reshape(B, S, C, N)
    qx = np.asarray(query_x, dtype=np.float32).reshape(B, S, C, N)
    qn = np.sqrt(np.sum(qx * qx, axis=2))                   # [B,S,N]
    qhat = qx / np.maximum(qn, EPS)[:, :, None, :]
    sn = np.sqrt(np.sum(sx * sx, axis=2))                   # [B,S,N]
    rs = 1.0 / np.maximum(sn, EPS)
    sup = _pack(sx)
    sup0 = np.ascontiguousarray(sup[:, 0:64, :])
    sup1 = np.ascontiguousarray(sup[:, 64:128, :])
    return sup0, sup1, _pack(qhat), rs


def _make_in_maps(support_x, query_x):
    sup0, sup1, qh, rs = _prep(support_x, query_x)
    return [{"support0": sup0[b], "support1": sup1[b], "query": qh[b]}
            for b in range(B)], rs


def kernel(support_x, query_x, **_unused):
    in_maps, rs = _make_in_maps(support_x, query_x)
    nc = _get_nc()
    res = run_bass_kernel_spmd(nc, in_maps, core_ids=list(range(B)))

    i = np.arange(N)
    m, r = i // 128, i % 128
    out = np.empty((B, S, N), dtype=np.float32)
    for b in range(B):
        cm = np.asarray(res.results[b]["cmax"]).reshape(128, SP, 3)
        out[b] = cm[r, :, m].T[0:S] * rs[b]
    return np.ascontiguousarray(out)


# revision 16
# speedup vs baseline: 1.0104x; 1.0104x over previous
"""Trainium2 Bass kernel v7 for nn_Middle_Moudle_v3 (retrieval_knn).

Per episode (b, s): cosine similarity of every support spatial C-vector
against every query spatial C-vector, max over query positions.

  support_x, query_x: [8, 75, 64, 19, 19] fp32  ->  out [8, 75, 361] fp32

Data-parallel over batch (8 episodes -> 8 cores).

Design (v7):
  - Query is normalized ON HOST (fp32) and shipped as q-hat bf16; support
    norms are applied on host after the kernel (scale commutes with max).
    No on-device norm pipeline at all.
  - HAM fix: K=64 half-array matmuls never register as PE activity (v2
    trace: clock stuck at 1.2 GHz the whole main phase).  The host ships
    the support TWICE with the opposite partition half zeroed (sup0:
    even pairs in partitions 0-63, zeros below; sup1: odd pairs in
    64-127, zeros above).  Every main matmul is then a genuine K=128
    full-array op -> HAM stays at 2.4 GHz (153ns/MM).
  - Drain split tuned from trace so both PSUM-capable engines run
    concurrently at equal load: per 12-pair span, P%4==0 direct-reduces
    on DVE from PSUM (1490ns); the other 9 are ACT-copied to SBUF bf16
    (1163ns) and max-folded on DVE (overlap-fold ladder, ~650ns/pair).
  - Input DMA spread over three HW queues (sync/tensor/vector issue) so
    each carries ~3.5MB and the first tile group lands early; ACT's
    queue carries no DMA triggers.
"""
import numpy as np
import ml_dtypes

import concourse.bass as bass
import concourse.mybir as mybir
import concourse.tile as tile
from concourse.bass_utils import run_bass_kernel_spmd

F32 = mybir.dt.float32
BF16 = mybir.dt.bfloat16
B = 8
S = 75
C = 64
N = 361
N2 = 362
SP = 76        # padded pairs
NT = 38        # two-pair tiles
NP = 75        # computed pairs (pad pair 75 skipped)
CHUNKS = [(0, 128), (128, 128), (256, 105)]
DGROUPS = [(0, 3), (3, 5), (8, 6), (14, 6), (20, 6), (26, 6), (32, 6)]
HEAT = 26      # K=128 heater matmuls to warm the PE clock during input DMA
EPS = 1e-8

SPANP = 12     # pairs per span
NSPAN = 6
# span drain mixes: B = 4 direct + 8 fold (DVE-heavy, early where DVE
# idles on DMA), A = 3 + 9 (balanced), C = 2 + 10 (ACT-heavy, late where
# ACT would idle); pairs 72-74 fold via ACT into its idle tail ('M').
import os as _os
SPAN_TYPES = list(_os.environ.get('KSPANS', 'BBAAAC'))
END_FOLD = _os.environ.get('KENDFOLD', '1') == '1'

_ws_ctr = [0]


def _split_multi_waits(nc):
    """Move all-but-one sync wait of each instruction onto injected
    InstEventSemaphore instructions (standalone sequencer waits)."""
    for f in nc.m.functions:
        for bb in f.blocks:
            insts = list(bb.instructions)
            out = []
            changed = False
            for ins in insts:
                si = ins.sync_info
                if si is not None and len(si.on_wait) > 1:
                    waits = list(si.on_wait)
                    for w in waits[:-1]:
                        _ws_ctr[0] += 1
                        ev = mybir.InstEventSemaphore(
                            name=f"wsplit_{_ws_ctr[0]}",
                            engine=ins.engine,
                            sync_info=mybir.SyncInfo(on_wait=[w], on_update=[]),
                        )
                        out.append(ev)
                    ins.sync_info = mybir.SyncInfo(
                        on_wait=[waits[-1]], on_update=list(si.on_update)
                    )
                    changed = True
                out.append(ins)
            if changed:
                bb.instructions = out


def _build_nc():
    win_np = np.zeros((128, 152), dtype=np.float32)
    win_np[:, 74] = 1.0

    nc = bass.Bass(target_bir_lowering=False)
    sup0_d = nc.dram_tensor("support0", [64, NT * N2], BF16,
                            kind="ExternalInput")
    sup1_d = nc.dram_tensor("support1", [64, NT * N2], BF16,
                            kind="ExternalInput")
    qry_d = nc.dram_tensor("query", [128, NT * N2], BF16, kind="ExternalInput")
    cmax_d = nc.dram_tensor("cmax", [128, 3 * SP], F32, kind="ExternalOutput")
    win_d = nc.inline_tensor(win_np, name="win")

    mx = mybir.AluOpType.max

    with tile.TileContext(nc) as tc:
        with tc.tile_pool(name="inp", bufs=1) as inp, \
             tc.tile_pool(name="work", bufs=1) as work, \
             tc.tile_pool(name="fbp", bufs=2) as fbp, \
             tc.tile_pool(name="psd", bufs=1, space="PSUM") as psd:

            # heater weights (K=128) land first; heat the PE clock
            # while the big input DMAs stream in.
            win32 = work.tile([128, 152], F32)
            nc.sync.dma_start(win32[:], win_d[:])
            win_sb = work.tile([128, 152], BF16)
            nc.vector.tensor_copy(win_sb[:], win32[:])

            sup0 = inp.tile([128, NT, N2], BF16)
            sup1 = inp.tile([128, NT, N2], BF16)
            qt = inp.tile([128, NT, N2], BF16)

            # data halves by DMA; dead halves zeroed per-group on the
            # otherwise-idle GPSIMD engine (input bytes: 10.6 -> 7.0 MB).
            for (g0, T) in DGROUPS:
                a, b = g0 * N2, (g0 + T) * N2
                nc.gpsimd.memset(sup0[64:128, g0:g0 + T, :], 0.0)
                nc.gpsimd.memset(sup1[0:64, g0:g0 + T, :], 0.0)
                nc.sync.dma_start(sup0[0:64, g0:g0 + T, :], sup0_d[:, a:b])
                nc.scalar.dma_start(sup1[64:128, g0:g0 + T, :],
                                    sup1_d[:, a:b])
                nc.sync.dma_start(qt[:, g0:g0 + T, :], qry_d[:, a:b])

            colmax = work.tile([128, SP, 3], F32)

            # --- 8-bank chunk-slot rotation: chunk (P, m) lands in bank
            # (3P+m) %% 8, giving ~2.67 pairs of PSUM pipeline depth.
            dots8 = psd.tile([128, 8, 512], F32)

            # --- heater: full-array K=128 matmuls into bank 0 (WAW-
            # serialized ahead of pair 0's first chunk)
            for h in range(HEAT):
                nc.tensor.matmul(dots8[:, 0, 0:152], win_sb[:, 0:128],
                                 win_sb[:, 0:152], start=True, stop=True)

            pend = []

            def fold_chain(fb, s0, ns):
                """Overlap-fold ladder on fb slots [s0, s0+ns)."""
                steps = []
                for (hi, w) in ((180, 182), (90, 92), (46, 46), (22, 24)):
                    steps.append(lambda hi=hi, w=w: nc.vector.tensor_tensor(
                        out=fb[:, s0:s0 + ns, :, 0:w],
                        in0=fb[:, s0:s0 + ns, :, 0:w],
                        in1=fb[:, s0:s0 + ns, :, hi:hi + w], op=mx))
                return steps

            def tails(fb, bi, styp):
                """Strided tail reduces mapping fb slots to colmax."""
                P0 = SPANP * bi
                steps = []

                def tail(s0_, sstep, nf, pP0, pstride):
                    cv = colmax[:, pP0, :]
                    out_ap = bass.AP(tensor=cv.tensor, offset=cv.offset,
                                     ap=[list(cv.ap[0]), [3 * pstride, nf],
                                         [1, 3]])
                    v = fb[:, s0_:s0_ + 1, :, 0:24]
                    in_ap = bass.AP(tensor=v.tensor, offset=v.offset,
                                    ap=[list(v.ap[0]), [sstep * 3 * N2, nf],
                                        [N2, 3], [1, 24]])
                    nc.vector.tensor_reduce(
                        out_ap, in_ap, axis=mybir.AxisListType.X, op=mx)
                if styp == 'A':
                    spec = [(3 * t, 1, 3, P0 + 1 + t, 4) for t in range(3)]
                elif styp == 'B':
                    spec = [(4 * t, 1, 4, P0 + 1 + t, 3) for t in range(2)]
                elif styp == 'C':
                    spec = [(i, 5, 2, P0 + 1 + i, 6) for i in range(5)]
                else:   # M: pairs 72..74 in slots 0..2
                    spec = [(0, 1, 3, 72, 1)]
                for (a, b, c, dd, ee) in spec:
                    steps.append(lambda a=a, b=b, c=c, dd=dd, ee=ee:
                                 tail(a, b, c, dd, ee))
                return steps

            fb = None
            for j in range(NT):
                for e in range(2):
                    P = 2 * j + e
                    if P >= NP:
                        continue
                    supE = sup0 if e == 0 else sup1
                    s0 = (3 * P) % 8
                    for m, (off, mc) in enumerate(CHUNKS):
                        nc.tensor.matmul(
                            dots8[0:mc, (s0 + m) % 8, 0:N2],
                            supE[:, j, off:off + mc],
                            qt[:, j, 0:N2],
                            start=True, stop=True,
                        )
                    pj = P % SPANP
                    bi = P // SPANP
                    styp = (SPAN_TYPES[bi] if bi < NSPAN
                            else ('M' if END_FOLD else 'D'))
                    if pj == 0 and bi < NSPAN:
                        fb = fbp.tile([128, 10, 3, N2], BF16, tag="fb",
                                      name=f"fb{bi}")
                    if styp == 'M' and P == 72:
                        fb = fbp.tile([128, 10, 3, N2], BF16, tag="fb",
                                      name="fb6")
                    if styp == 'A':
                        direct = (pj % 4 == 0)
                        slot = -1 if direct else 3 * (pj % 4 - 1) + pj // 4
                    elif styp == 'B':
                        direct = (pj % 3 == 0)
                        slot = -1 if direct else 4 * (pj % 3 - 1) + pj // 3
                    elif styp == 'C':
                        direct = (pj in (0, 6))
                        slot = -1 if direct else (pj - 1 if pj < 6 else pj - 2)
                    elif styp == 'M':
                        direct = False
                        slot = P - 72
                    else:
                        direct = True
                        slot = -1
                    nw = min(3, 8 - s0)   # chunks before the bank wrap
                    if direct:
                        if nw == 3:
                            nc.vector.tensor_reduce(
                                colmax[:, P, :], dots8[:, s0:s0 + 3, 0:N2],
                                axis=mybir.AxisListType.X, op=mx)
                        else:
                            nc.vector.tensor_reduce(
                                colmax[:, P, 0:nw],
                                dots8[:, s0:s0 + nw, 0:N2],
                                axis=mybir.AxisListType.X, op=mx)
                            nc.vector.tensor_reduce(
                                colmax[:, P, nw:3],
                                dots8[:, 0:3 - nw, 0:N2],
                                axis=mybir.AxisListType.X, op=mx)
                    else:
                        if nw == 3:
                            nc.scalar.copy(fb[:, slot, :, :],
                                           dots8[:, s0:s0 + 3, 0:N2])
                        else:
                            nc.scalar.copy(fb[:, slot, 0:nw, :],
                                           dots8[:, s0:s0 + nw, 0:N2])
                            nc.scalar.copy(fb[:, slot, nw:3, :],
                                           dots8[:, 0:3 - nw, 0:N2])
                        if pj == SPANP - 1 and styp == 'A':
                            pend.extend(fold_chain(fb, 0, 9))
                            pend.extend(tails(fb, bi, 'A'))
                        elif pj == SPANP - 1 and styp == 'B':
                            pend.extend(fold_chain(fb, 0, 8))
                            pend.extend(tails(fb, bi, 'B'))
                        elif pj == SPANP - 1 and styp == 'C':
                            pend.extend(fold_chain(fb, 0, 10))
                            pend.extend(tails(fb, bi, 'C'))
                        elif styp == 'M' and P == NP - 1:
                            pend.extend(fold_chain(fb, 0, 3))
                            pend.extend(tails(fb, 6, 'M'))
                    if P == 48:
                        # all colmax[:, 0:36] writers are issued by now
                        nc.sync.dma_start(cmax_d[:, 0:108],
                                          colmax[:, 0:36, :])
                    if P == 72:
                        nc.sync.dma_start(cmax_d[:, 108:180],
                                          colmax[:, 36:60, :])
                    if pend and (pj >= 2 or bi >= NSPAN - 1):
                        pend.pop(0)()
                        if pend:
                            pend.pop(0)()
            for fn in pend:
                fn()

            nc.sync.dma_start(cmax_d[:, 180:3 * SP], colmax[:, 60:SP, :])

    _split_multi_waits(nc)
    return nc


_NC_CACHE = None


def _get_nc():
    global _NC_CACHE
    if _NC_CACHE is None:
        _NC_CACHE = _build_nc()
    return _NC_CACHE


def _pack(x):
    """[B, S, C, N] fp32 -> [B, 128, NT*N2] bf16 (pad dup)."""
    x = np.concatenate([x, x[:, S - 1:S]], axis=1)          # pair 75 = dup 74
    x = np.concatenate([x, x[:, :, :, N - 1:N]], axis=3)    # col 361 = dup 360
    x = x.reshape(B, NT, 2, C, N2).transpose(0, 2, 3, 1, 4)  # [B,2,C,NT,N2]
    x = x.reshape(B, 128, NT * N2)
    return np.ascontiguousarray(x.astype(ml_dtypes.bfloat16))


def _prep(support_x, query_x):
    sx = np.asarray(support_x, dtype=np.float32).reshape(B, S, C, N)
    qx = np.asarray(query_x, dtype=np.float32).reshape(B, S, C, N)
    qn = np.sqrt(np.sum(qx * qx, axis=2))                   # [B,S,N]
    qhat = qx / np.maximum(qn, EPS)[:, :, None, :]
    sn = np.sqrt(np.sum(sx * sx, axis=2))                   # [B,S,N]
    rs = 1.0 / np.maximum(sn, EPS)
    sup = _pack(sx)
    sup0 = np.ascontiguousarray(sup[:, 0:64, :])
    sup1 = np.ascontiguousarray(sup[:, 64:128, :])
    return sup0, sup1, _pack(qhat), rs


def _make_in_maps(support_x, query_x):
    sup0, sup1, qh, rs = _prep(support_x, query_x)
    return [{"support0": sup0[b], "support1": sup1[b], "query": qh[b]}
            for b in range(B)], rs


def kernel(support_x, query_x, **_unused):
    in_maps, rs = _make_in_maps(support_x, query_x)
    nc = _get_nc()
    res = run_bass_kernel_spmd(nc, in_maps, core_ids=list(range(B)))

    i = np.arange(N)
    m, r = i // 128, i % 128
    out = np.empty((B, S, N), dtype=np.float32)
    for b in range(B):
        cm = np.asarray(res.results[b]["cmax"]).reshape(128, SP, 3)
        out[b] = cm[r, :, m].T[0:S] * rs[b]
    return np.ascontiguousarray(out)
